# revision 17
# baseline (speedup 1.0000x reference)
"""Linformer self-attention on 8 Trainium2 NeuronCores.

Problem (hardcoded shapes): x [4,4096,1024] f32; per batch:
  q = scale*(x@Wq); kv = x@Wkv; keys/values compressed 4096->256 via
  proj_k/proj_v; 16-head attention (dh=64, k=256); out @ Wproj + bproj.

Sharding: 8 cores = 4 batches x 2 head-groups (8 heads / 512 cols each).
Each core computes a partial [4096,1024] output (Wproj row-split); host
sums the pair and adds bias.

Per-core dataflow (all matmuls use out = lhsT.T @ rhs, K<=128 partitions):
  A : xcxvT[1024,512] = x.T @ [proj_k|proj_v]      (contract n, x natural)
  A2: kprojT[512,256] = Wk_g.T @ xcT ; vproj[256,512] = xvT.T @ Wv_g
  B : qT[512,4096] = Wq_g.T @ xT    (xT provided by host, plain DMA)
  C : per (head,fc k-chunk): scoresT[128,512] -> exp (Act) -> pexp bf16
  S : per (n-chunk, head): sums[n,1] = pexp.T @ ones  (N=1 matmuls)
  D : po[n, 8*64] = pexp.T @ vproj_h per head; normalize via DVE
      tensor_tensor with per-head recip broadcast -> o bf16
  T : oT via one batched DMA transpose per [128,512] tile
  E : out[n,1024] = oT.T-chunks @ Wproj_g, bf16 store via gpsimd SWDGE

B(nb+1), E(nb-1), D(nb) are hand-interleaved in the PE stream per
n-block so Act exp latency hides under PE matmuls.
"""

import os
import numpy as np

import concourse.bass as bass
import concourse.mybir as mybir
import concourse.tile as tile
from concourse import bacc
from concourse.bass_utils import run_bass_kernel_spmd

P = 128
N, D, K, DG, DH = 4096, 1024, 256, 512, 64
NB = 8                    # n-blocks of 512
HL = 8                    # heads per core
F32 = mybir.dt.float32

MMDT_NAME = os.environ.get("LINF_MMDT", "bfloat16")
MMDT = getattr(mybir.dt, MMDT_NAME)
Exp = mybir.ActivationFunctionType.Exp

_cache = {}


def build_nc():
    nc = bacc.Bacc(None, target_bir_lowering=False, debug=False)

    x_d = nc.dram_tensor("x", [N, D], MMDT, kind="ExternalInput")
    xt_d = nc.dram_tensor("xt", [D, N], MMDT, kind="ExternalInput")
    pkv_d = nc.dram_tensor("projkv", [N, 2 * K], MMDT, kind="ExternalInput")
    wq_d = nc.dram_tensor("wq", [D, DG], MMDT, kind="ExternalInput")
    wk_d = nc.dram_tensor("wk", [D, DG], MMDT, kind="ExternalInput")
    wv_d = nc.dram_tensor("wv", [D, DG], MMDT, kind="ExternalInput")
    wp_d = nc.dram_tensor("wproj", [DG, D], MMDT, kind="ExternalInput")
    out_d = nc.dram_tensor("out", [N, D], MMDT, kind="ExternalOutput")

    with tile.TileContext(nc) as tc:
        from contextlib import ExitStack
        with ExitStack() as ctx:
            res = ctx.enter_context(tc.tile_pool(name="res", bufs=1))
            ones_sb = res.tile([P, 1], MMDT, tag="ones")
            nc.vector.memset(ones_sb[:], 1.0)

            wq_sb = res.tile([P, 8 * DG], MMDT, tag="wq")
            wk_sb = res.tile([P, 8 * DG], MMDT, tag="wk")
            wv_sb = res.tile([P, 8 * DG], MMDT, tag="wv")
            wproj_sb = res.tile([P, 4 * D], MMDT, tag="wproj")
            kprojT_sb = res.tile([P, 4 * K], MMDT, tag="kprojT")
            vproj_sb = res.tile([P, 2 * DG], MMDT, tag="vproj")
            xcxv_sb = res.tile([P, 8 * 2 * K], MMDT, tag="xcxv")

            # rolling pools for the merged loop
            xtp = ctx.enter_context(tc.tile_pool(name="xtp", bufs=4))
            qtp = ctx.enter_context(tc.tile_pool(name="qtp", bufs=2))
            pexp_p = ctx.enter_context(tc.tile_pool(name="pexp", bufs=2))
            op_ = ctx.enter_context(tc.tile_pool(name="op", bufs=8))
            otp = ctx.enter_context(tc.tile_pool(name="otp", bufs=8))
            outp = ctx.enter_context(tc.tile_pool(name="outp", bufs=3))
            rcp = ctx.enter_context(tc.tile_pool(name="rcp", bufs=2))

            def load_w(dst, src, nchunk, w):
                # dst[p, c*w + j] = src[c*128 + p, j]
                nc.sync.dma_start(
                    out=dst[:].rearrange("p (c j) -> p c j", c=nchunk),
                    in_=src[:, :].rearrange("(c p) j -> p c j", p=P))

            def load_xt(nb):
                xt = xtp.tile([P, 8 * DG], MMDT, tag="xt", name=f"xt{nb}")
                nc.sync.dma_start(
                    out=xt[:].rearrange("p (d j) -> p d j", d=8),
                    in_=xt_d[:, nb * DG:(nb + 1) * DG]
                        .rearrange("(d p) j -> p d j", p=P))
                return xt

            # ---------------- Phase A ----------------
            with ExitStack() as actx:
                xin = actx.enter_context(tc.tile_pool(name="xin", bufs=3))
                pa_ctx = ExitStack()
                pa = pa_ctx.enter_context(tc.tile_pool(name="pa", bufs=1, space="PSUM"))
                accs = [pa.tile([P, 2 * K], F32, tag=f"pa{dd}", name=f"pa{dd}")
                        for dd in range(8)]
                # First chunk loads alone (small, fast) so PE starts ASAP;
                # all weight/xt loads go after the 8 batches — the DMA
                # transfer path is a serial resource and phase A is tight.
                for b4 in range(8):
                    if b4 == 0:
                        # per-chunk loads so the PE can start after ~2.3us
                        x4 = xin.tile([P, 4 * D], MMDT, tag="x4")
                        kv4 = xin.tile([P, 4 * 2 * K], MMDT, tag="kv4")
                        for c in range(4):
                            nc.sync.dma_start(out=x4[:, c * D:(c + 1) * D],
                                              in_=x_d[c * P:(c + 1) * P, :])
                            nc.sync.dma_start(
                                out=kv4[:, c * 2 * K:(c + 1) * 2 * K],
                                in_=pkv_d[c * P:(c + 1) * P, :])
                    else:
                        x4 = xin.tile([P, 4 * D], MMDT, tag="x4")
                        kv4 = xin.tile([P, 4 * 2 * K], MMDT, tag="kv4")
                        nc.sync.dma_start(
                            out=x4[:].rearrange("p (c j) -> p c j", c=4),
                            in_=x_d[b4 * 512:(b4 + 1) * 512, :]
                                .rearrange("(c p) j -> p c j", p=P))
                        nc.sync.dma_start(
                            out=kv4[:].rearrange("p (c j) -> p c j", c=4),
                            in_=pkv_d[b4 * 512:(b4 + 1) * 512, :]
                                .rearrange("(c p) j -> p c j", p=P))
                    for c in range(4):
                        nn = b4 * 4 + c
                        for dd in range(8):
                            nc.tensor.matmul(
                                accs[dd][:],
                                lhsT=x4[:, c * D + dd * P: c * D + (dd + 1) * P],
                                rhs=kv4[:, c * 2 * K:(c + 1) * 2 * K],
                                start=(nn == 0), stop=(nn == 31))
                # ordered by first use: wk/wv (A2), wq+xt0 (B prologue),
                # wproj (E(0)), xt1 (B(1))
                load_w(wk_sb, wk_d, 8, DG)
                load_w(wv_sb, wv_d, 8, DG)
                load_w(wq_sb, wq_d, 8, DG)
                xt_tiles = {0: load_xt(0)}
                load_w(wproj_sb, wp_d, 4, D)
                xt_tiles[1] = load_xt(1)
                for dd in range(8):
                    eng = nc.vector if dd % 2 else nc.scalar
                    if dd % 2:
                        nc.vector.tensor_copy(
                            xcxv_sb[:, dd * 2 * K:(dd + 1) * 2 * K], accs[dd][:])
                    else:
                        nc.scalar.copy(
                            out=xcxv_sb[:, dd * 2 * K:(dd + 1) * 2 * K],
                            in_=accs[dd][:])

                # Phase A2 — release the A accumulators' banks first
                pa_ctx.close()
                pa2 = actx.enter_context(tc.tile_pool(name="pa2", bufs=4, space="PSUM"))
                for jc in range(4):
                    acc = pa2.tile([P, K], F32, tag="kpj")
                    for dd in range(8):
                        nc.tensor.matmul(
                            acc[:],
                            lhsT=wk_sb[:, dd * DG + jc * P: dd * DG + (jc + 1) * P],
                            rhs=xcxv_sb[:, dd * 2 * K: dd * 2 * K + K],
                            start=(dd == 0), stop=(dd == 7))
                    if jc % 2:
                        nc.scalar.copy(out=kprojT_sb[:, jc * K:(jc + 1) * K],
                                       in_=acc[:])
                    else:
                        nc.vector.tensor_copy(kprojT_sb[:, jc * K:(jc + 1) * K],
                                              acc[:])
                for fc in range(2):
                    acc2 = pa2.tile([P, DG], F32, tag="vpj")
                    for dd in range(8):
                        nc.tensor.matmul(
                            acc2[:],
                            lhsT=xcxv_sb[:, dd * 2 * K + K + fc * P:
                                         dd * 2 * K + K + (fc + 1) * P],
                            rhs=wv_sb[:, dd * DG:(dd + 1) * DG],
                            start=(dd == 0), stop=(dd == 7))
                    if fc:
                        nc.scalar.copy(out=vproj_sb[:, fc * DG:(fc + 1) * DG],
                                       in_=acc2[:])
                    else:
                        nc.vector.tensor_copy(vproj_sb[:, fc * DG:(fc + 1) * DG],
                                              acc2[:])

            # ---------------- merged loop pools (PSUM) ----------------
            scp = ctx.enter_context(tc.tile_pool(name="scp", bufs=2, space="PSUM"))
            accp = ctx.enter_context(tc.tile_pool(name="accp", bufs=3, space="PSUM"))
            pop = ctx.enter_context(tc.tile_pool(name="pop", bufs=2, space="PSUM"))
            smp = ctx.enter_context(tc.tile_pool(name="smp", bufs=1, space="PSUM"))

            def b_block(xt, qt, jc):
                accq = accp.tile([P, DG], F32, tag="acc")
                for dd in range(8):
                    nc.tensor.matmul(
                        accq[:],
                        lhsT=wq_sb[:, dd * DG + jc * P: dd * DG + (jc + 1) * P],
                        rhs=xt[:, dd * DG:(dd + 1) * DG],
                        start=(dd == 0), stop=(dd == 7))
                nc.vector.tensor_copy(qt[:, jc * DG:(jc + 1) * DG], accq[:])

            def sc_block(qt, h, pexps):
                jc, p0 = h // 2, (h % 2) * DH
                for fc in range(2):
                    st = scp.tile([P, DG], F32, tag="sc")
                    nc.tensor.matmul(
                        st[:],
                        lhsT=kprojT_sb[p0:p0 + DH,
                                       jc * K + fc * P: jc * K + (fc + 1) * P],
                        rhs=qt[p0:p0 + DH, jc * DG:(jc + 1) * DG],
                        start=True, stop=True)
                    pexp = pexp_p.tile([P, DG], MMDT, tag=f"px{h}_{fc}")
                    nc.scalar.activation(pexp[:], st[:], Exp)
                    pexps[(h, fc)] = pexp

            def d_block(nb, pexps, sp, recips):
                o_tiles = []
                for nn2 in range(4):
                    po = pop.tile([P, DG], F32, tag="po")
                    for h in range(HL):
                        for fc in range(2):
                            px = pexps[(h, fc)]
                            nc.tensor.matmul(
                                po[:, h * DH:(h + 1) * DH],
                                lhsT=px[:, nn2 * P:(nn2 + 1) * P],
                                rhs=vproj_sb[:, fc * DG + h * DH:
                                             fc * DG + (h + 1) * DH],
                                start=(fc == 0), stop=(fc == 1))
                            nc.tensor.matmul(
                                sp[:, nn2 * HL + h: nn2 * HL + h + 1],
                                lhsT=px[:, nn2 * P:(nn2 + 1) * P],
                                rhs=ones_sb[:],
                                start=(fc == 0), stop=(fc == 1))
                    nc.vector.reciprocal(
                        recips[:, nn2 * HL:(nn2 + 1) * HL],
                        sp[:, nn2 * HL:(nn2 + 1) * HL])
                    o_t = op_.tile([P, DG], MMDT, tag="o", name=f"o{nb}_{nn2}")
                    nc.vector.tensor_tensor(
                        out=o_t[:].rearrange("p (h j) -> p h j", h=HL),
                        in0=po[:].rearrange("p (h j) -> p h j", h=HL),
                        in1=recips[:, nn2 * HL:(nn2 + 1) * HL]
                            .broadcast_to([P, HL, DH]),
                        op=mybir.AluOpType.mult)
                    ot = otp.tile([P, DG], MMDT, tag="ot", name=f"ot{nb}_{nn2}")
                    nc.scalar.dma_start_transpose(
                        out=ot[:].rearrange("p (c j) -> p c j", c=4),
                        in_=o_t[:])
                    o_tiles.append(ot)
                return o_tiles

            def e_block(nb, ots):
                last = nb == NB - 1
                for nn2 in range(4):
                    ci = nb * 4 + nn2
                    ot = ots[nn2]
                    outsb = outp.tile([P, D], MMDT, tag="outsb")
                    for half in range(2):
                        pe_acc = accp.tile([P, DG], F32, tag="acc")
                        for jc2 in range(4):
                            nc.tensor.matmul(
                                pe_acc[:],
                                lhsT=ot[:, jc2 * P:(jc2 + 1) * P],
                                rhs=wproj_sb[:, jc2 * D + half * DG:
                                             jc2 * D + (half + 1) * DG],
                                start=(jc2 == 0), stop=(jc2 == 3))
                        # alternate engines so neither serializes the chain
                        if (nn2 + half) % 2 or (last and nn2 == 3):
                            nc.scalar.copy(
                                out=outsb[:, half * DG:(half + 1) * DG],
                                in_=pe_acc[:])
                        else:
                            nc.vector.tensor_copy(
                                outsb[:, half * DG:(half + 1) * DG], pe_acc[:])
                    if last and nn2 >= 2:
                        # HWDGE store has lower fixed latency than SWDGE;
                        # use it for the final stores on the critical tail
                        nc.sync.dma_start(out=out_d[ci * P:(ci + 1) * P, :],
                                          in_=outsb[:])
                    else:
                        nc.gpsimd.dma_start(out=out_d[ci * P:(ci + 1) * P, :],
                                            in_=outsb[:])

            # ---------------- prologue: B(0) ----------------
            qts = {0: qtp.tile([P, 4 * DG], MMDT, tag="qt", name="qt0")}
            for jc in range(4):
                b_block(xt_tiles[0], qts[0], jc)

            # ---------------- merged loop ----------------
            xt_tiles[2] = load_xt(2)
            prev_ots = None
            for nb in range(NB):
                if nb + 3 < NB:
                    xt_tiles[nb + 3] = load_xt(nb + 3)
                pexps = {}
                sp = smp.tile([P, 4 * HL], F32, tag="sums")
                recips = rcp.tile([P, 4 * HL], F32, tag="recips")
                have_b = nb + 1 < NB
                if have_b:
                    qts[nb + 1] = qtp.tile([P, 4 * DG], MMDT, tag="qt",
                                           name=f"qt{nb + 1}")
                # interleave scores(nb) with B(nb+1) on the PE stream
                for h in range(HL):
                    sc_block(qts[nb], h, pexps)
                    if have_b and h % 2 == 1 and h // 2 < 4:
                        b_block(xt_tiles[nb + 1], qts[nb + 1], h // 2)
                if prev_ots is not None:
                    e_block(nb - 1, prev_ots)
                prev_ots = d_block(nb, pexps, sp, recips)
            e_block(NB - 1, prev_ots)
    nc.compile()
    return nc


def _np_mm(a):
    return np.ascontiguousarray(np.asarray(a), dtype=mybir.dt.np(MMDT))


def kernel(x, Wq, Wkv, Wproj, bproj, proj_k, proj_v):
    x = np.asarray(x)
    Wq, Wkv, Wproj = np.asarray(Wq), np.asarray(Wkv), np.asarray(Wproj)
    bproj, proj_k, proj_v = np.asarray(bproj), np.asarray(proj_k), np.asarray(proj_v)

    if "nc" not in _cache:
        _cache["nc"] = build_nc()
    nc = _cache["nc"]

    scale = np.float32(DH ** -0.5)
    projkv = _np_mm(np.concatenate([proj_k, proj_v], axis=1))
    in_maps = []
    for c in range(8):
        b, g = c // 2, c % 2
        cols = slice(g * DG, (g + 1) * DG)
        xb = _np_mm(x[b])
        in_maps.append({
            "x": xb,
            "xt": np.ascontiguousarray(xb.T),
            "projkv": projkv,
            "wq": _np_mm(scale * Wq[:, cols]),
            "wk": _np_mm(Wkv[:, :D][:, cols]),
            "wv": _np_mm(Wkv[:, D:][:, cols]),
            "wproj": _np_mm(Wproj[cols, :]),
        })
    res = run_bass_kernel_spmd(nc, in_maps, list(range(8)),
                               trace=bool(os.environ.get("LINF_TRACE")))
    _cache["last_result"] = res
    outs = [np.asarray(r["out"], dtype=np.float32) for r in res.results]
    full = np.stack([outs[2 * b] + outs[2 * b + 1] for b in range(4)])
    full = full + np.asarray(bproj, np.float32)
    return full.astype(np.float32)
